# revision 26
# baseline (speedup 1.0000x reference)
"""Trainium2 Bass kernel for MultiHeadSelfAttentionModelV1.

Model (per batch row):
    e   = emb_table[x]                      # [S, E]
    Q/K/V = e @ W* + b*                     # [S, E], split into H heads of Dh
    S_h = Q_h K_h^T / sqrt(Dh)              # [S, S] per head
    P_h = softmax(S_h, axis=-1)
    ctx = concat_h(P_h V_h) @ Wo + bo       # [S, E]
    out = max_tokens(ctx) @ Wc + bc         # [OUT]

Sharding: pure data parallel over batch. B == n_cores == 8, so each core
computes one batch row end-to-end; no collectives. The full emb table is
replicated to each core's DRAM; the on-device gather only reads S rows.

Per-core layout choices (SBUF is [partition, free]):
 - e is gathered token-major via indirect DMA, then PE-transposed to
   eT [feat, tok] which both projection matmul operands need.
 - Q^T, K^T are produced feature-major ([E, S]) so per-head slices are
   directly the matmul operands for scores; V is produced token-major
   ([S, E]) because the PV matmul wants it as the stationary operand.
 - Scores are computed transposed, S^T [k_tok, q_tok], two heads packed
   into one PE pass via tile_position row tiling (contraction dim is
   Dh=64). exp(x/8) runs on ACT straight out of PSUM.
 - V is augmented with a ones column per head, so the PV matmul's PSUM
   accumulator row 64 collects sum_k P = the softmax denominator.
 - The denominator row is partition-broadcast via DMA, reciprocal'd on
   DVE, and folded into the ctx PSUM->SBUF eviction multiply.
 - Output projection is feature-major, so max-pool over tokens is a DVE
   free-axis reduce straight from PSUM (bias commutes with max and is
   added to the [E]-sized pooled vector instead).

All matmuls run as float32r (full PE rate, ~tf32 precision); everything
else is fp32.
"""

import os
import sys

import numpy as np

if "/opt/trn_rl_repo" not in sys.path:
    sys.path.insert(0, "/opt/trn_rl_repo")

import concourse.bass as bass
import concourse.bacc as bacc
import concourse.tile as tile
from concourse import mybir
from concourse.masks import make_identity

F32 = mybir.dt.float32
F32R = mybir.dt.float32r
BF16 = mybir.dt.bfloat16
I32 = mybir.dt.int32
EXP = mybir.ActivationFunctionType.Exp
ADD = mybir.AluOpType.add
# Schraudolph bit-trick exp for the DVE-offloaded score chunks:
# exp(s/8) ~= bitcast_f32(int32(s * SCH_A8 + SCH_B)); ~1.5% mean rel err,
# near-zero bias, which the softmax normalization mostly cancels.
SCH_A8 = 12102203.161561485 * 0.125
SCH_B = 1064866805.0

MAX = mybir.AluOpType.max
MULT = mybir.AluOpType.mult

B = 8
E = 512
H = 8
DH = 64
OUT = 10
N_CORES = 8


def r(ap):
    return ap.bitcast(F32R)


def build(S=2048, VOCAB=50257):
    """Build the per-core Bass program (same program on all 8 cores)."""
    nc = bacc.Bacc()

    NT = S // 128   # 128-token tiles
    NJ = S // 512   # 512-token chunks
    NE = E // 128   # 128-feature chunks (4)

    # ~3/16 of the exp chunks run on the otherwise-idle DVE via the
    # Schraudolph bit trick; the rest on ACT.
    dve_exp_i = set(range(1, NT, 4))

    xi = nc.declare_dram_parameter("xi", [128, NT], I32, isOutput=False)
    emb = nc.declare_dram_parameter("emb", [VOCAB, E], F32, isOutput=False)
    wq = nc.declare_dram_parameter("wq", [E, E], BF16, isOutput=False)
    wk = nc.declare_dram_parameter("wk", [E, E], BF16, isOutput=False)
    wv = nc.declare_dram_parameter("wv", [E, E], BF16, isOutput=False)
    wo = nc.declare_dram_parameter("wo", [E, E], BF16, isOutput=False)
    wc = nc.declare_dram_parameter("wc", [E, OUT], F32, isOutput=False)
    bq = nc.declare_dram_parameter("bq", [128, NE], F32, isOutput=False)
    bk = nc.declare_dram_parameter("bk", [128, NE], F32, isOutput=False)
    bo = nc.declare_dram_parameter("bo", [128, NE], F32, isOutput=False)
    bv = nc.declare_dram_parameter("bv", [1, E], BF16, isOutput=False)
    bc = nc.declare_dram_parameter("bc", [OUT, 1], F32, isOutput=False)
    out = nc.declare_dram_parameter("out", [OUT, 1], F32, isOutput=True)

    with tile.TileContext(nc) as tc:
        with (
            tc.tile_pool(name="consts", bufs=1) as consts,
            tc.tile_pool(name="qkT", bufs=1) as qkT_pool,
            tc.tile_pool(name="vaug", bufs=1) as vaug_pool,
            tc.tile_pool(name="ctxT", bufs=1) as ctxT_pool,
            tc.tile_pool(name="fin", bufs=1) as fin_pool,
        ):
            # ---- constants ----
            idx_sb = consts.tile([128, NT], I32, tag="idx")
            nc.sync.dma_start(out=idx_sb, in_=xi[:, :])
            ident = consts.tile([128, 128], F32, tag="ident")
            make_identity(nc, ident)
            wo_sb = [consts.tile([128, E], BF16, tag=f"wo{k}", name=f"wo{k}") for k in range(NE)]
            for k in range(NE):
                nc.sync.dma_start(out=wo_sb[k], in_=wo[k * 128:(k + 1) * 128, :])
            wc_sb = [consts.tile([128, OUT], F32, tag=f"wc{k}", name=f"wc{k}") for k in range(NE)]
            for k in range(NE):
                nc.sync.dma_start(out=wc_sb[k], in_=wc[k * 128:(k + 1) * 128, :])
            bq_sb = consts.tile([128, NE], F32, tag="bq")
            nc.sync.dma_start(out=bq_sb, in_=bq[:, :])
            bk_sb = consts.tile([128, NE], F32, tag="bk")
            nc.sync.dma_start(out=bk_sb, in_=bk[:, :])
            bo_sb = consts.tile([128, NE], F32, tag="bo")
            nc.sync.dma_start(out=bo_sb, in_=bo[:, :])
            bv_sb = consts.tile([1, E], BF16, tag="bv")
            nc.sync.dma_start(out=bv_sb, in_=bv[:, :])
            bc_sb = consts.tile([OUT, 1], F32, tag="bc")
            nc.sync.dma_start(out=bc_sb, in_=bc[:, :])
            ones_row = consts.tile([1, 128], BF16, tag="ones")
            nc.vector.memset(ones_row, 1.0)

            # persistent activations
            QT = [qkT_pool.tile([128, S], BF16, tag=f"qt{k}", name=f"qt{k}") for k in range(NE)]
            KT = [qkT_pool.tile([128, S], BF16, tag=f"kt{k}", name=f"kt{k}") for k in range(NE)]
            # V in 128-wide per-head blocks: [V_h (64) | ones (1) | zeros(63)]
            # so the PV matmul is a full-width M=128 stationary; PSUM row 64
            # accumulates the softmax denominator.
            VA = [vaug_pool.tile([128, H * 128], BF16, tag=f"va{t}", name=f"va{t}")
                  for t in range(NT)]
            CT = [ctxT_pool.tile([128, S], BF16, tag=f"ct{k}", name=f"ct{k}") for k in range(NE)]

            # ================= phase A+B: gather, eT, QKV =================
            with (
                tc.tile_pool(name="projw", bufs=1) as projw,
                tc.tile_pool(name="eT", bufs=1) as eT_pool,
                tc.tile_pool(name="enat", bufs=3) as enat_pool,
                tc.tile_pool(name="tps", bufs=2, space="PSUM") as tps,
                tc.tile_pool(name="qkvps", bufs=4, space="PSUM") as qkvps,
            ):
                wq_sb = [projw.tile([128, E], BF16, tag=f"wq{k}", name=f"wq{k}") for k in range(NE)]
                wk_sb = [projw.tile([128, E], BF16, tag=f"wk{k}", name=f"wk{k}") for k in range(NE)]
                wv_sb = [projw.tile([128, E], BF16, tag=f"wv{k}", name=f"wv{k}") for k in range(NE)]
                for k in range(NE):
                    nc.sync.dma_start(out=wq_sb[k], in_=wq[k * 128:(k + 1) * 128, :])
                    nc.sync.dma_start(out=wk_sb[k], in_=wk[k * 128:(k + 1) * 128, :])
                    nc.sync.dma_start(out=wv_sb[k], in_=wv[k * 128:(k + 1) * 128, :])

                eT = [eT_pool.tile([128, S], BF16, tag=f"et{k}", name=f"et{k}") for k in range(NE)]

                for t in range(NT):
                    e_nat = enat_pool.tile([128, E], F32)
                    nc.gpsimd.indirect_dma_start(
                        out=e_nat[:],
                        out_offset=None,
                        in_=emb[:, :],
                        in_offset=bass.IndirectOffsetOnAxis(
                            ap=idx_sb[:, t:t + 1], axis=0
                        ),
                    )
                    for f in range(NE):
                        tp = tps.tile([128, 128], F32)
                        nc.tensor.transpose(
                            out=tp[:], in_=e_nat[:, f * 128:(f + 1) * 128],
                            identity=ident[:],
                        )
                        nc.vector.tensor_copy(
                            out=eT[f][:, t * 128:(t + 1) * 128], in_=tp[:]
                        )

                # Q^T / K^T feature-major
                for name, w_sb, b_sb in (
                    ("q", wq_sb, bq_sb),
                    ("k", wk_sb, bk_sb),
                ):  # noqa
                    for m in range(NE):
                        for j in range(NJ):
                            ps = qkvps.tile([128, 512], F32, tag="qkv")
                            for kk in range(NE):
                                nc.tensor.matmul(
                                    out=ps[:],
                                    lhsT=(w_sb[kk][:, m * 128:(m + 1) * 128]),
                                    rhs=(eT[kk][:, j * 512:(j + 1) * 512]),
                                    start=(kk == 0),
                                    stop=(kk == NE - 1),
                                )
                            dstT = QT[m] if name == "q" else KT[m]
                            nc.vector.tensor_scalar_add(
                                out=dstT[:, j * 512:(j + 1) * 512],
                                in0=ps[:],
                                scalar1=b_sb[:, m:m + 1],
                            )

                # V token-major, augmented with a ones column per head
                for t in range(NT):
                    ps = qkvps.tile([128, 512], F32, tag="qkv")
                    for kk in range(NE):
                        nc.tensor.matmul(
                            out=ps[:],
                            lhsT=(eT[kk][:, t * 128:(t + 1) * 128]),
                            rhs=(wv_sb[kk][:]),
                            start=(kk == 0),
                            stop=False,
                        )
                    nc.tensor.matmul(
                        out=ps[:],
                        lhsT=(ones_row[:]),
                        rhs=(bv_sb[:]),
                        start=False,
                        stop=True,
                    )
                    va = VA[t].rearrange("p (h c) -> p h c", c=128)
                    nc.vector.tensor_copy(
                        out=va[:, :, 0:DH],
                        in_=ps[:].rearrange("p (h c) -> p h c", c=DH),
                    )
                    nc.vector.memset(va[:, :, DH:DH + 1], 1.0)
                    nc.vector.memset(va[:, :, DH + 1:128], 0.0)

            # ================= phase C: attention =================
            with (
                tc.tile_pool(name="pt", bufs=5) as pt_pool,
                tc.tile_pool(name="rep", bufs=6) as rep_pool,
                tc.tile_pool(name="sps", bufs=3, space="PSUM") as sps,
                tc.tile_pool(name="ctxps", bufs=2, space="PSUM") as ctxps,
            ):
                for hp in range(H // 2):
                    for j in range(NJ):
                        ctx_e = ctxps.tile([128, 512], F32, tag="ctx")
                        ctx_o = ctxps.tile([128, 512], F32, tag="ctx")
                        for i in range(NT):
                            # Both heads' score tiles share one PSUM slot, so
                            # the row-tiled pair becomes issue-ready together
                            # and runs concurrently in the PE array.
                            stile = sps.tile([128, 1024], F32,
                                             tag="s", name="stile")
                            nc.tensor.matmul(
                                out=stile[:, 0:512],
                                lhsT=(KT[hp][0:64, i * 128:(i + 1) * 128]),
                                rhs=(QT[hp][0:64, j * 512:(j + 1) * 512]),
                                start=True, stop=True,
                                tile_position=(0, 0),
                            )
                            nc.tensor.matmul(
                                out=stile[:, 512:1024],
                                lhsT=(KT[hp][64:128, i * 128:(i + 1) * 128]),
                                rhs=(QT[hp][64:128, j * 512:(j + 1) * 512]),
                                start=True, stop=True,
                                tile_position=(64, 0),
                            )
                            pt = pt_pool.tile([128, 1024], BF16,
                                              tag="pt", name="pt")
                            if i in dve_exp_i:
                                t1 = pt_pool.tile([128, 1024], F32,
                                                  tag="t1", name="t1")
                                nc.vector.tensor_scalar(
                                    out=t1[:], in0=stile[:],
                                    scalar1=SCH_A8, scalar2=SCH_B,
                                    op0=MULT, op1=ADD,
                                )
                                t2 = pt_pool.tile([128, 1024], I32,
                                                  tag="t2", name="t2")
                                nc.vector.tensor_copy(out=t2[:], in_=t1[:])
                                nc.vector.tensor_copy(
                                    out=pt[:], in_=t2[:].bitcast(F32)
                                )
                            else:
                                nc.scalar.activation(
                                    out=pt[:], in_=stile[:],
                                    func=EXP, scale=0.125,
                                )
                            for ctx, off, h in (
                                (ctx_e, 0, 2 * hp),
                                (ctx_o, 512, 2 * hp + 1),
                            ):
                                nc.tensor.matmul(
                                    out=ctx[:],
                                    lhsT=(VA[i][:, h * 128:(h + 1) * 128]),
                                    rhs=(pt[:, off:off + 512]),
                                    start=(i == 0),
                                    stop=(i == NT - 1),
                                    skip_group_check=True,
                                )
                        # normalize: ctx[0:64] / ctx[64] -> ctxT slice. Evict
                        # the PSUM accumulator to SBUF immediately (frees the
                        # bank), then run the denominator chain out of SBUF:
                        # DVE reciprocal is ~6 cyc/lane-element, so compute
                        # the 512 reciprocals in a [128, 4] partition-parallel
                        # layout, then fan the row back out with the gpsimd
                        # partition-broadcast.
                        for ctx, h in ((ctx_e, 2 * hp), (ctx_o, 2 * hp + 1)):
                            ctx_sb = rep_pool.tile([DH + 1, 512], F32, tag="ctx_sb")
                            nc.vector.tensor_copy(
                                out=ctx_sb[:], in_=ctx[0:DH + 1, :]
                            )
                            l128 = rep_pool.tile([128, 4], F32, tag="l128")
                            nc.gpsimd.dma_start(out=l128[:], in_=ctx_sb[DH:DH + 1, :])
                            nc.vector.reciprocal(out=l128[:], in_=l128[:])
                            rrow = rep_pool.tile([1, 512], F32, tag="rrow")
                            nc.gpsimd.dma_start(out=rrow[:], in_=l128[:])
                            rep = rep_pool.tile([64, 512], F32, tag="rep")
                            nc.gpsimd.partition_broadcast(rep[:], rrow[:])
                            nc.vector.tensor_tensor(
                                out=CT[hp][(h % 2) * 64:(h % 2) * 64 + 64,
                                           j * 512:(j + 1) * 512],
                                in0=ctx_sb[0:DH, :],
                                in1=rep[:],
                                op=MULT,
                            )

            # ================= phase D: out proj, maxpool, classifier ====
            with (
                tc.tile_pool(name="mx", bufs=4) as mx_pool,
                tc.tile_pool(name="ovps", bufs=4, space="PSUM") as ovps,
                tc.tile_pool(name="clsps", bufs=1, space="PSUM") as clsps,
            ):
                pooled = [fin_pool.tile([128, 1], F32, tag=f"pool{m}", name=f"pool{m}")
                          for m in range(NE)]
                for m in range(NE):
                    acc = mx_pool.tile([128, 1], F32, tag="acc")
                    for j in range(NJ):
                        ps = ovps.tile([128, 512], F32, tag="ov")
                        for kk in range(NE):
                            nc.tensor.matmul(
                                out=ps[:],
                                lhsT=(wo_sb[kk][:, m * 128:(m + 1) * 128]),
                                rhs=(CT[kk][:, j * 512:(j + 1) * 512]),
                                start=(kk == 0),
                                stop=(kk == NE - 1),
                            )
                        if j == 0:
                            nc.vector.reduce_max(
                                out=acc[:], in_=ps[:], axis=mybir.AxisListType.X
                            )
                        else:
                            tmp = mx_pool.tile([128, 1], F32, tag="tmp")
                            nc.vector.reduce_max(
                                out=tmp[:], in_=ps[:], axis=mybir.AxisListType.X
                            )
                            nc.vector.tensor_tensor(
                                out=acc[:], in0=acc[:], in1=tmp[:], op=MAX
                            )
                    nc.vector.tensor_scalar_add(
                        out=pooled[m][:], in0=acc[:], scalar1=bo_sb[:, m:m + 1]
                    )

                cls = clsps.tile([OUT, 1], F32, tag="cls")
                for kk in range(NE):
                    nc.tensor.matmul(
                        out=cls[:],
                        lhsT=wc_sb[kk][:],
                        rhs=pooled[kk][:],
                        start=(kk == 0),
                        stop=(kk == NE - 1),
                    )
                logits = fin_pool.tile([OUT, 1], F32, tag="logits")
                nc.vector.tensor_scalar_add(
                    out=logits[:], in0=cls[:], scalar1=bc_sb[:]
                )
                nc.sync.dma_start(out=out[:, :], in_=logits[:])

    nc.finalize()  # Bacc: legalizes sync waits + allocates registers
    return nc


def make_in_maps(inputs):
    """Shard the full inputs into per-core (per-batch-row) input dicts."""
    import ml_dtypes

    bf16 = ml_dtypes.bfloat16
    x = np.asarray(inputs["x"]).astype(np.int32)          # [B, S]
    S = x.shape[1]
    emb = np.ascontiguousarray(np.asarray(inputs["emb_table"], dtype=np.float32))
    shared = {
        "emb": emb,
        "wq": np.ascontiguousarray(np.asarray(inputs["Wq"]).astype(bf16)),
        "wk": np.ascontiguousarray(np.asarray(inputs["Wk"]).astype(bf16)),
        "wv": np.ascontiguousarray(np.asarray(inputs["Wv"]).astype(bf16)),
        "wo": np.ascontiguousarray(np.asarray(inputs["Wo"]).astype(bf16)),
        "wc": np.ascontiguousarray(np.asarray(inputs["Wc"], dtype=np.float32)),
        "bq": np.ascontiguousarray(
            np.asarray(inputs["bq"], dtype=np.float32).reshape(4, 128).T),
        "bk": np.ascontiguousarray(
            np.asarray(inputs["bk"], dtype=np.float32).reshape(4, 128).T),
        "bo": np.ascontiguousarray(
            np.asarray(inputs["bo"], dtype=np.float32).reshape(4, 128).T),
        "bv": np.ascontiguousarray(
            np.asarray(inputs["bv"]).astype(bf16).reshape(1, E)),
        "bc": np.ascontiguousarray(
            np.asarray(inputs["bc"], dtype=np.float32).reshape(OUT, 1)),
    }
    in_maps = []
    for c in range(x.shape[0]):
        xi = np.ascontiguousarray(x[c].reshape(S // 128, 128).T)  # [128, NT]
        in_maps.append({"xi": xi, **shared})
    return in_maps


_NC_CACHE = {}


def get_nc(S=2048, VOCAB=50257):
    key = (S, VOCAB)
    if key not in _NC_CACHE:
        _NC_CACHE[key] = build(S, VOCAB)
    return _NC_CACHE[key]


def run(inputs, trace=False):
    from concourse.bass_utils import run_bass_kernel_spmd

    nc = get_nc()
    in_maps = make_in_maps(inputs)
    res = run_bass_kernel_spmd(
        nc, in_maps, list(range(N_CORES)), trace=trace
    )
    outs = np.stack(
        [res.results[c]["out"].reshape(OUT) for c in range(N_CORES)]
    ).astype(np.float32)
    return outs, res


def kernel(**inputs):
    outs, _ = run(inputs, trace=False)
    return outs


# revision 27
# speedup vs baseline: 1.0780x; 1.0780x over previous
"""Trainium2 Bass kernel for MultiHeadSelfAttentionModelV1.

Model (per batch row):
    e   = emb_table[x]                      # [S, E]
    Q/K/V = e @ W* + b*                     # [S, E], split into H heads of Dh
    S_h = Q_h K_h^T / sqrt(Dh)              # [S, S] per head
    P_h = softmax(S_h, axis=-1)
    ctx = concat_h(P_h V_h) @ Wo + bo       # [S, E]
    out = max_tokens(ctx) @ Wc + bc         # [OUT]

Sharding: pure data parallel over batch. B == n_cores == 8, so each core
computes one batch row end-to-end; no collectives. The full emb table is
replicated to each core's DRAM; the on-device gather only reads S rows.

Per-core layout choices (SBUF is [partition, free]):
 - e is gathered token-major via indirect DMA, then PE-transposed to
   eT [feat, tok] which both projection matmul operands need.
 - Q^T, K^T are produced feature-major ([E, S]) so per-head slices are
   directly the matmul operands for scores; V is produced token-major
   ([S, E]) because the PV matmul wants it as the stationary operand.
 - Scores are computed transposed, S^T [k_tok, q_tok], two heads packed
   into one PE pass via tile_position row tiling (contraction dim is
   Dh=64). exp(x/8) runs on ACT straight out of PSUM.
 - V is augmented with a ones column per head, so the PV matmul's PSUM
   accumulator row 64 collects sum_k P = the softmax denominator.
 - The denominator row is partition-broadcast via DMA, reciprocal'd on
   DVE, and folded into the ctx PSUM->SBUF eviction multiply.
 - Output projection is feature-major, so max-pool over tokens is a DVE
   free-axis reduce straight from PSUM (bias commutes with max and is
   added to the [E]-sized pooled vector instead).

All matmuls run as float32r (full PE rate, ~tf32 precision); everything
else is fp32.
"""

import os
import sys

import numpy as np

if "/opt/trn_rl_repo" not in sys.path:
    sys.path.insert(0, "/opt/trn_rl_repo")

import concourse.bass as bass
import concourse.bacc as bacc
import concourse.tile as tile
from concourse import mybir
from concourse.masks import make_identity

F32 = mybir.dt.float32
F32R = mybir.dt.float32r
BF16 = mybir.dt.bfloat16
I32 = mybir.dt.int32
EXP = mybir.ActivationFunctionType.Exp
ADD = mybir.AluOpType.add
# Schraudolph bit-trick exp for the DVE-offloaded score chunks:
# exp(s/8) ~= bitcast_f32(int32(s * SCH_A8 + SCH_B)); ~1.5% mean rel err,
# near-zero bias, which the softmax normalization mostly cancels.
SCH_A8 = 12102203.161561485 * 0.125
SCH_B = 1064866805.0

MAX = mybir.AluOpType.max
MULT = mybir.AluOpType.mult

B = 8
E = 512
H = 8
DH = 64
OUT = 10
N_CORES = 8


def r(ap):
    return ap.bitcast(F32R)


def build(S=2048, VOCAB=50257):
    """Build the per-core Bass program (same program on all 8 cores)."""
    nc = bacc.Bacc()

    NT = S // 128   # 128-token tiles
    NJ = S // 512   # 512-token chunks
    NE = E // 128   # 128-feature chunks (4)

    # ~3/16 of the exp chunks run on the otherwise-idle DVE via the
    # Schraudolph bit trick; the rest on ACT.
    dve_exp_i = set(range(1, NT, 5))

    xi = nc.declare_dram_parameter("xi", [128, NT], I32, isOutput=False)
    emb = nc.declare_dram_parameter("emb", [VOCAB, E], F32, isOutput=False)
    wq = nc.declare_dram_parameter("wq", [E, E], BF16, isOutput=False)
    wk = nc.declare_dram_parameter("wk", [E, E], BF16, isOutput=False)
    wv = nc.declare_dram_parameter("wv", [E, E], BF16, isOutput=False)
    wo = nc.declare_dram_parameter("wo", [E, E], BF16, isOutput=False)
    wc = nc.declare_dram_parameter("wc", [E, OUT], F32, isOutput=False)
    bq = nc.declare_dram_parameter("bq", [128, NE], F32, isOutput=False)
    bk = nc.declare_dram_parameter("bk", [128, NE], F32, isOutput=False)
    bo = nc.declare_dram_parameter("bo", [128, NE], F32, isOutput=False)
    bv = nc.declare_dram_parameter("bv", [1, E], BF16, isOutput=False)
    bc = nc.declare_dram_parameter("bc", [OUT, 1], F32, isOutput=False)
    out = nc.declare_dram_parameter("out", [OUT, 1], F32, isOutput=True)

    with tile.TileContext(nc) as tc:
        with (
            tc.tile_pool(name="consts", bufs=1) as consts,
            tc.tile_pool(name="qkT", bufs=1) as qkT_pool,
            tc.tile_pool(name="vaug", bufs=1) as vaug_pool,
            tc.tile_pool(name="ctxT", bufs=1) as ctxT_pool,
            tc.tile_pool(name="fin", bufs=1) as fin_pool,
        ):
            # ---- constants ----
            idx_sb = consts.tile([128, NT], I32, tag="idx")
            nc.sync.dma_start(out=idx_sb, in_=xi[:, :])
            ident = consts.tile([128, 128], F32, tag="ident")
            make_identity(nc, ident)
            wo_sb = [consts.tile([128, E], BF16, tag=f"wo{k}", name=f"wo{k}") for k in range(NE)]
            for k in range(NE):
                nc.sync.dma_start(out=wo_sb[k], in_=wo[k * 128:(k + 1) * 128, :])
            wc_sb = [consts.tile([128, OUT], F32, tag=f"wc{k}", name=f"wc{k}") for k in range(NE)]
            for k in range(NE):
                nc.sync.dma_start(out=wc_sb[k], in_=wc[k * 128:(k + 1) * 128, :])
            bq_sb = consts.tile([128, NE], F32, tag="bq")
            nc.sync.dma_start(out=bq_sb, in_=bq[:, :])
            bk_sb = consts.tile([128, NE], F32, tag="bk")
            nc.sync.dma_start(out=bk_sb, in_=bk[:, :])
            bo_sb = consts.tile([128, NE], F32, tag="bo")
            nc.sync.dma_start(out=bo_sb, in_=bo[:, :])
            bv_sb = consts.tile([1, E], BF16, tag="bv")
            nc.sync.dma_start(out=bv_sb, in_=bv[:, :])
            bc_sb = consts.tile([OUT, 1], F32, tag="bc")
            nc.sync.dma_start(out=bc_sb, in_=bc[:, :])
            ones_row = consts.tile([1, 128], BF16, tag="ones")
            nc.vector.memset(ones_row, 1.0)

            # persistent activations
            QT = [qkT_pool.tile([128, S], BF16, tag=f"qt{k}", name=f"qt{k}") for k in range(NE)]
            KT = [qkT_pool.tile([128, S], BF16, tag=f"kt{k}", name=f"kt{k}") for k in range(NE)]
            # V in 128-wide per-head blocks: [V_h (64) | ones (1) | zeros(63)]
            # so the PV matmul is a full-width M=128 stationary; PSUM row 64
            # accumulates the softmax denominator.
            VA = [vaug_pool.tile([128, H * 128], BF16, tag=f"va{t}", name=f"va{t}")
                  for t in range(NT)]
            CT = [ctxT_pool.tile([128, S], BF16, tag=f"ct{k}", name=f"ct{k}") for k in range(NE)]

            # ================= phase A+B: gather, eT, QKV =================
            with (
                tc.tile_pool(name="projw", bufs=1) as projw,
                tc.tile_pool(name="eT", bufs=1) as eT_pool,
                tc.tile_pool(name="enat", bufs=3) as enat_pool,
                tc.tile_pool(name="tps", bufs=2, space="PSUM") as tps,
                tc.tile_pool(name="qkvps", bufs=4, space="PSUM") as qkvps,
            ):
                wq_sb = [projw.tile([128, E], BF16, tag=f"wq{k}", name=f"wq{k}") for k in range(NE)]
                wk_sb = [projw.tile([128, E], BF16, tag=f"wk{k}", name=f"wk{k}") for k in range(NE)]
                wv_sb = [projw.tile([128, E], BF16, tag=f"wv{k}", name=f"wv{k}") for k in range(NE)]
                for k in range(NE):
                    nc.sync.dma_start(out=wq_sb[k], in_=wq[k * 128:(k + 1) * 128, :])
                    nc.sync.dma_start(out=wk_sb[k], in_=wk[k * 128:(k + 1) * 128, :])
                    nc.sync.dma_start(out=wv_sb[k], in_=wv[k * 128:(k + 1) * 128, :])

                eT = [eT_pool.tile([128, S], BF16, tag=f"et{k}", name=f"et{k}") for k in range(NE)]

                for t in range(NT):
                    e_nat = enat_pool.tile([128, E], F32)
                    nc.gpsimd.indirect_dma_start(
                        out=e_nat[:],
                        out_offset=None,
                        in_=emb[:, :],
                        in_offset=bass.IndirectOffsetOnAxis(
                            ap=idx_sb[:, t:t + 1], axis=0
                        ),
                    )
                    for f in range(NE):
                        tp = tps.tile([128, 128], F32)
                        nc.tensor.transpose(
                            out=tp[:], in_=e_nat[:, f * 128:(f + 1) * 128],
                            identity=ident[:],
                        )
                        nc.vector.tensor_copy(
                            out=eT[f][:, t * 128:(t + 1) * 128], in_=tp[:]
                        )

                # Q^T / K^T feature-major
                for name, w_sb, b_sb in (
                    ("q", wq_sb, bq_sb),
                    ("k", wk_sb, bk_sb),
                ):  # noqa
                    for m in range(NE):
                        for j in range(NJ):
                            ps = qkvps.tile([128, 512], F32, tag="qkv")
                            for kk in range(NE):
                                nc.tensor.matmul(
                                    out=ps[:],
                                    lhsT=(w_sb[kk][:, m * 128:(m + 1) * 128]),
                                    rhs=(eT[kk][:, j * 512:(j + 1) * 512]),
                                    start=(kk == 0),
                                    stop=(kk == NE - 1),
                                )
                            dstT = QT[m] if name == "q" else KT[m]
                            nc.vector.tensor_scalar_add(
                                out=dstT[:, j * 512:(j + 1) * 512],
                                in0=ps[:],
                                scalar1=b_sb[:, m:m + 1],
                            )

                # V token-major, augmented with a ones column per head
                for t in range(NT):
                    ps = qkvps.tile([128, 512], F32, tag="qkv")
                    for kk in range(NE):
                        nc.tensor.matmul(
                            out=ps[:],
                            lhsT=(eT[kk][:, t * 128:(t + 1) * 128]),
                            rhs=(wv_sb[kk][:]),
                            start=(kk == 0),
                            stop=False,
                        )
                    nc.tensor.matmul(
                        out=ps[:],
                        lhsT=(ones_row[:]),
                        rhs=(bv_sb[:]),
                        start=False,
                        stop=True,
                    )
                    va = VA[t].rearrange("p (h c) -> p h c", c=128)
                    nc.vector.tensor_copy(
                        out=va[:, :, 0:DH],
                        in_=ps[:].rearrange("p (h c) -> p h c", c=DH),
                    )
                    nc.vector.memset(va[:, :, DH:DH + 1], 1.0)
                    nc.vector.memset(va[:, :, DH + 1:128], 0.0)

            # ================= phase C: attention =================
            with (
                tc.tile_pool(name="pt", bufs=5) as pt_pool,
                tc.tile_pool(name="rep", bufs=6) as rep_pool,
                tc.tile_pool(name="sps", bufs=3, space="PSUM") as sps,
                tc.tile_pool(name="ctxps", bufs=2, space="PSUM") as ctxps,
            ):
                for hp in range(H // 2):
                    for j in range(NJ):
                        ctx_e = ctxps.tile([128, 512], F32, tag="ctx")
                        ctx_o = ctxps.tile([128, 512], F32, tag="ctx")
                        for i in range(NT):
                            # Both heads' score tiles share one PSUM slot, so
                            # the row-tiled pair becomes issue-ready together
                            # and runs concurrently in the PE array.
                            stile = sps.tile([128, 1024], F32,
                                             tag="s", name="stile")
                            nc.tensor.matmul(
                                out=stile[:, 0:512],
                                lhsT=(KT[hp][0:64, i * 128:(i + 1) * 128]),
                                rhs=(QT[hp][0:64, j * 512:(j + 1) * 512]),
                                start=True, stop=True,
                                tile_position=(0, 0),
                            )
                            nc.tensor.matmul(
                                out=stile[:, 512:1024],
                                lhsT=(KT[hp][64:128, i * 128:(i + 1) * 128]),
                                rhs=(QT[hp][64:128, j * 512:(j + 1) * 512]),
                                start=True, stop=True,
                                tile_position=(64, 0),
                            )
                            pt = pt_pool.tile([128, 1024], BF16,
                                              tag="pt", name="pt")
                            if i in dve_exp_i:
                                t1 = pt_pool.tile([128, 1024], F32,
                                                  tag="t1", name="t1")
                                nc.vector.tensor_scalar(
                                    out=t1[:], in0=stile[:],
                                    scalar1=SCH_A8, scalar2=SCH_B,
                                    op0=MULT, op1=ADD,
                                )
                                t2 = pt_pool.tile([128, 1024], I32,
                                                  tag="t2", name="t2")
                                nc.vector.tensor_copy(out=t2[:], in_=t1[:])
                                nc.vector.tensor_copy(
                                    out=pt[:], in_=t2[:].bitcast(F32)
                                )
                            else:
                                nc.scalar.activation(
                                    out=pt[:], in_=stile[:],
                                    func=EXP, scale=0.125,
                                )
                            for ctx, off, h in (
                                (ctx_e, 0, 2 * hp),
                                (ctx_o, 512, 2 * hp + 1),
                            ):
                                nc.tensor.matmul(
                                    out=ctx[:],
                                    lhsT=(VA[i][:, h * 128:(h + 1) * 128]),
                                    rhs=(pt[:, off:off + 512]),
                                    start=(i == 0),
                                    stop=(i == NT - 1),
                                    skip_group_check=True,
                                )
                        # normalize: ctx[0:64] / ctx[64] -> ctxT slice. Evict
                        # the PSUM accumulator to SBUF immediately (frees the
                        # bank), then run the denominator chain out of SBUF:
                        # DVE reciprocal is ~6 cyc/lane-element, so compute
                        # the 512 reciprocals in a [128, 4] partition-parallel
                        # layout, then fan the row back out with the gpsimd
                        # partition-broadcast.
                        for ctx, h in ((ctx_e, 2 * hp), (ctx_o, 2 * hp + 1)):
                            ctx_sb = rep_pool.tile([DH + 1, 512], F32, tag="ctx_sb")
                            nc.vector.tensor_copy(
                                out=ctx_sb[:], in_=ctx[0:DH + 1, :]
                            )
                            l128 = rep_pool.tile([128, 4], F32, tag="l128")
                            nc.sync.dma_start(out=l128[:], in_=ctx_sb[DH:DH + 1, :])
                            nc.vector.reciprocal(out=l128[:], in_=l128[:])
                            rrow = rep_pool.tile([1, 512], F32, tag="rrow")
                            nc.sync.dma_start(out=rrow[:], in_=l128[:])
                            rep = rep_pool.tile([64, 512], F32, tag="rep")
                            nc.gpsimd.partition_broadcast(rep[:], rrow[:])
                            nc.vector.tensor_tensor(
                                out=CT[hp][(h % 2) * 64:(h % 2) * 64 + 64,
                                           j * 512:(j + 1) * 512],
                                in0=ctx_sb[0:DH, :],
                                in1=rep[:],
                                op=MULT,
                            )

            # ================= phase D: out proj, maxpool, classifier ====
            with (
                tc.tile_pool(name="mx", bufs=4) as mx_pool,
                tc.tile_pool(name="ovps", bufs=4, space="PSUM") as ovps,
                tc.tile_pool(name="clsps", bufs=1, space="PSUM") as clsps,
            ):
                pooled = [fin_pool.tile([128, 1], F32, tag=f"pool{m}", name=f"pool{m}")
                          for m in range(NE)]
                for m in range(NE):
                    acc = mx_pool.tile([128, 1], F32, tag="acc")
                    for j in range(NJ):
                        ps = ovps.tile([128, 512], F32, tag="ov")
                        for kk in range(NE):
                            nc.tensor.matmul(
                                out=ps[:],
                                lhsT=(wo_sb[kk][:, m * 128:(m + 1) * 128]),
                                rhs=(CT[kk][:, j * 512:(j + 1) * 512]),
                                start=(kk == 0),
                                stop=(kk == NE - 1),
                            )
                        if j == 0:
                            nc.vector.reduce_max(
                                out=acc[:], in_=ps[:], axis=mybir.AxisListType.X
                            )
                        else:
                            tmp = mx_pool.tile([128, 1], F32, tag="tmp")
                            nc.vector.reduce_max(
                                out=tmp[:], in_=ps[:], axis=mybir.AxisListType.X
                            )
                            nc.vector.tensor_tensor(
                                out=acc[:], in0=acc[:], in1=tmp[:], op=MAX
                            )
                    nc.vector.tensor_scalar_add(
                        out=pooled[m][:], in0=acc[:], scalar1=bo_sb[:, m:m + 1]
                    )

                cls = clsps.tile([OUT, 1], F32, tag="cls")
                for kk in range(NE):
                    nc.tensor.matmul(
                        out=cls[:],
                        lhsT=wc_sb[kk][:],
                        rhs=pooled[kk][:],
                        start=(kk == 0),
                        stop=(kk == NE - 1),
                    )
                logits = fin_pool.tile([OUT, 1], F32, tag="logits")
                nc.vector.tensor_scalar_add(
                    out=logits[:], in0=cls[:], scalar1=bc_sb[:]
                )
                nc.sync.dma_start(out=out[:, :], in_=logits[:])

    nc.finalize()  # Bacc: legalizes sync waits + allocates registers
    return nc


def make_in_maps(inputs):
    """Shard the full inputs into per-core (per-batch-row) input dicts."""
    import ml_dtypes

    bf16 = ml_dtypes.bfloat16
    x = np.asarray(inputs["x"]).astype(np.int32)          # [B, S]
    S = x.shape[1]
    emb = np.ascontiguousarray(np.asarray(inputs["emb_table"], dtype=np.float32))
    shared = {
        "emb": emb,
        "wq": np.ascontiguousarray(np.asarray(inputs["Wq"]).astype(bf16)),
        "wk": np.ascontiguousarray(np.asarray(inputs["Wk"]).astype(bf16)),
        "wv": np.ascontiguousarray(np.asarray(inputs["Wv"]).astype(bf16)),
        "wo": np.ascontiguousarray(np.asarray(inputs["Wo"]).astype(bf16)),
        "wc": np.ascontiguousarray(np.asarray(inputs["Wc"], dtype=np.float32)),
        "bq": np.ascontiguousarray(
            np.asarray(inputs["bq"], dtype=np.float32).reshape(4, 128).T),
        "bk": np.ascontiguousarray(
            np.asarray(inputs["bk"], dtype=np.float32).reshape(4, 128).T),
        "bo": np.ascontiguousarray(
            np.asarray(inputs["bo"], dtype=np.float32).reshape(4, 128).T),
        "bv": np.ascontiguousarray(
            np.asarray(inputs["bv"]).astype(bf16).reshape(1, E)),
        "bc": np.ascontiguousarray(
            np.asarray(inputs["bc"], dtype=np.float32).reshape(OUT, 1)),
    }
    in_maps = []
    for c in range(x.shape[0]):
        xi = np.ascontiguousarray(x[c].reshape(S // 128, 128).T)  # [128, NT]
        in_maps.append({"xi": xi, **shared})
    return in_maps


_NC_CACHE = {}


def get_nc(S=2048, VOCAB=50257):
    key = (S, VOCAB)
    if key not in _NC_CACHE:
        _NC_CACHE[key] = build(S, VOCAB)
    return _NC_CACHE[key]


def run(inputs, trace=False):
    from concourse.bass_utils import run_bass_kernel_spmd

    nc = get_nc()
    in_maps = make_in_maps(inputs)
    res = run_bass_kernel_spmd(
        nc, in_maps, list(range(N_CORES)), trace=trace
    )
    outs = np.stack(
        [res.results[c]["out"].reshape(OUT) for c in range(N_CORES)]
    ).astype(np.float32)
    return outs, res


def kernel(**inputs):
    outs, _ = run(inputs, trace=False)
    return outs
